# revision 1
# baseline (speedup 1.0000x reference)
"""Trainium2 Bass kernel for a 2-layer IndRNN (adding-problem head).

Computation (matches the reference):
    pre1 = x @ W1.T + b1                    # [B,T,H], D=2
    h1   = scan over t: h = relu(pre1_t + u1*h)   (all steps kept)
    pre2 = h1 @ W2.T + b2                   # [B,T,H]
    h2T  = scan over t: h = relu(pre2_t + u2*h)   (last step only)
    out  = h2T @ Wf.T + bf                  # [B]

Sharding: data-parallel over batch across 8 NeuronCores (32 batch each).
All intermediates stay on-chip per core (no DRAM spills, no collectives).

Per-core layout is channel-major: [c_lo(128 partitions), (c_hi(4), t, b(32))].
Each scan advances one timestep per Vector-engine instruction pair:

    TT :  z  = w_prev + p_t           (tensor_tensor add)
    STT:  w  = (z max 0) * u_tile     (scalar_tensor_tensor: fused relu+mult)

w = u1*relu(z1) doubles as recurrence feedback and layer-2 matmul input
(the host folds 1/u1 into W2).  The two layers' scans are interleaved
instruction-by-instruction so independent chains overlap in the DVE
pipeline; layer 2 lags layer 1 by LAG chunks.

TensorE computes pre1 (K=2, weights stationary per c_hi block) and pre2
(W2 tiles stationary) in float32r (full PE rate at N=512); ScalarE drains
PSUM to SBUF with the per-channel bias fused into the activation.
"""

import os
import sys

for _p in ("/opt/trn_rl_repo", "/root/.axon_site", "/root/.axon_site/_ro/trn_rl_repo",
           "/root/.axon_site/_ro/pypackages"):
    if os.path.isdir(_p) and _p not in sys.path:
        sys.path.append(_p)

import numpy as np

B, T_FULL, D, H = 256, 2048, 2, 512
NCORES = 8
BL = B // NCORES          # 32 batch per core
TC = 32                   # timesteps per chunk
LAG = 2                   # layer-2 chunk lag behind layer 1

_COMPILED = {}


def _build(T):
    import contextlib
    from concourse import tile, bacc, mybir

    nchunks = T // TC
    nk = nchunks + LAG

    f32 = mybir.dt.float32
    f32r = mybir.dt.float32r
    Add = mybir.AluOpType.add
    Max = mybir.AluOpType.max
    Mult = mybir.AluOpType.mult
    Ident = mybir.ActivationFunctionType.Identity
    Relu = mybir.ActivationFunctionType.Relu

    CB = TC * BL            # free elems per (chunk, c_hi) = 1024
    NSUB = CB // 512        # 512-wide matmul subtiles per (chunk, c_hi)

    nc = bacc.Bacc("TRN2", target_bir_lowering=False, debug=False)

    x_d = nc.dram_tensor("x_sb", [2 * nchunks, CB], f32r, kind="ExternalInput").ap()
    w1_d = nc.dram_tensor("w1_rep", [2, 512], f32r, kind="ExternalInput").ap()
    b1_d = nc.dram_tensor("b1_col", [128, 4], f32, kind="ExternalInput").ap()
    u1_d = nc.dram_tensor("u1_tile", [128, 128], f32, kind="ExternalInput").ap()
    w2_d = nc.dram_tensor("w2t", [128, 2048], f32r, kind="ExternalInput").ap()
    b2_d = nc.dram_tensor("b2_col", [128, 4], f32, kind="ExternalInput").ap()
    u2_d = nc.dram_tensor("u2_tile", [128, 128], f32, kind="ExternalInput").ap()
    wf_d = nc.dram_tensor("wf_col", [128, 4], f32r, kind="ExternalInput").ap()
    out_d = nc.dram_tensor("out", [1, BL], f32, kind="ExternalOutput").ap()

    with tile.TileContext(nc) as tc:
        with contextlib.ExitStack() as ctx:
            consts = ctx.enter_context(tc.tile_pool(name="consts", bufs=1))
            p1_pool = ctx.enter_context(tc.tile_pool(name="p1", bufs=3))
            xs_pool = ctx.enter_context(tc.tile_pool(name="xs", bufs=4))
            w_pool = ctx.enter_context(tc.tile_pool(name="w", bufs=3))
            p2_pool = ctx.enter_context(tc.tile_pool(name="p2", bufs=4))
            st_pool = ctx.enter_context(tc.tile_pool(name="st", bufs=1))
            ps1 = ctx.enter_context(tc.tile_pool(name="ps1", bufs=2, space="PSUM"))
            ps2 = ctx.enter_context(tc.tile_pool(name="ps2", bufs=4, space="PSUM"))
            psf = ctx.enter_context(tc.tile_pool(name="psf", bufs=1, space="PSUM"))

            w1_sb = consts.tile([2, 512], f32r, name="w1_sbt")
            b1_sb = consts.tile([128, 4], f32, name="b1_sbt")
            u1_sb = consts.tile([128, 128], f32, name="u1_sbt")
            w2_sb = consts.tile([128, 2048], f32r, name="w2_sbt")
            b2_sb = consts.tile([128, 4], f32, name="b2_sbt")
            u2_sb = consts.tile([128, 128], f32, name="u2_sbt")
            wf_sb = consts.tile([128, 4], f32r, name="wf_sbt")
            for sb, dr in ((w1_sb, w1_d), (b1_sb, b1_d)):
                nc.gpsimd.dma_start(sb[:], dr[:])

            z1 = st_pool.tile([128, 128], f32, name="z1")
            z2 = st_pool.tile([128, 128], f32, name="z2")
            v2 = st_pool.tile([128, 128], f32, name="v2")
            zero = st_pool.tile([128, 128], f32, name="zero")
            nc.vector.memset(zero[:], 0.0)
            nc.vector.memset(v2[:], 0.0)

            p1_tiles, w_tiles, p2_tiles = {}, {}, {}

            def chunk_slot(tile_, i):
                # [128, 4, BL] view of a [128, 4*CB] chunk tile at step i
                return tile_[:].rearrange("p (c t b) -> p c t b", c=4, t=TC, b=BL)[:, :, i, :]

            def zview(t_):
                return t_[:].rearrange("p (c b) -> p c b", c=4, b=BL)

            def p1_matmul(k):
                xst = xs_pool.tile([2, CB], f32r, name=f"xst_{k}", tag="xst")
                nc.gpsimd.dma_start(xst[:], x_d[2 * k:2 * k + 2, :])
                pt = p1_pool.tile([128, 4 * CB], f32, name=f"p1t_{k}", tag="p1t")
                p1_tiles[k] = pt
                for sub in range(NSUB):
                    for c_hi in range(4):
                        ps = ps1.tile([128, 512], f32, name=f"p1ps_{k}_{c_hi}_{sub}", tag="p1ps")
                        lhsT = w1_sb[0:2, c_hi * 128:(c_hi + 1) * 128]
                        rhs = xst[0:2, sub * 512:(sub + 1) * 512]
                        nc.tensor.matmul(ps[:], lhsT, rhs,
                                         start=True, stop=True)
                        nc.scalar.activation(
                            pt[:, c_hi * CB + sub * 512: c_hi * CB + (sub + 1) * 512],
                            ps[:], Ident, bias=b1_sb[:, c_hi:c_hi + 1], scale=1.0)

            def w2_matmul(k):
                pt = p2_pool.tile([128, 4 * CB], f32, name=f"p2t_{k}", tag="p2t")
                p2_tiles[k] = pt
                wt = w_tiles[k]
                for g in range(4):
                    for sub in range(NSUB):
                        ps = ps2.tile([128, 512], f32, name=f"p2ps_{k}_{g}_{sub}", tag="p2ps")
                        for c in range(4):
                            lhsT = w2_sb[:, (c * 4 + g) * 128:(c * 4 + g + 1) * 128]
                            rhs = wt[:, c * CB + sub * 512: c * CB + (sub + 1) * 512]
                            nc.tensor.matmul(ps[:], lhsT, rhs,
                                             start=(c == 0), stop=(c == 3))
                        nc.scalar.activation(
                            pt[:, g * CB + sub * 512: g * CB + (sub + 1) * 512],
                            ps[:], Ident, bias=b2_sb[:, g:g + 1], scale=1.0)

            p1_matmul(0)
            for sb, dr in ((u1_sb, u1_d), (b2_sb, b2_d), (u2_sb, u2_d),
                           (wf_sb, wf_d), (w2_sb, w2_d)):
                nc.gpsimd.dma_start(sb[:], dr[:])
            if nchunks > 1:
                p1_matmul(1)

            for k in range(nk):
                if 1 <= k <= nchunks:
                    w2_matmul(k - 1)
                if k + 2 < nchunks:
                    p1_matmul(k + 2)

                l2k = k - LAG
                if k < nchunks:
                    wt = w_pool.tile([128, 4 * CB], f32r, name=f"wt_{k}", tag="wt")
                    w_tiles[k] = wt
                for i in range(TC):
                    if k < nchunks:
                        s = k * TC + i
                        if s == 0:
                            wprev = zview(zero)
                        elif i == 0:
                            wprev = chunk_slot(w_tiles[k - 1], TC - 1)
                        else:
                            wprev = chunk_slot(w_tiles[k], i - 1)
                        nc.vector.tensor_tensor(zview(z1), wprev,
                                                chunk_slot(p1_tiles[k], i), Add)
                    if l2k >= 0:
                        nc.vector.tensor_tensor(zview(z2), zview(v2),
                                                chunk_slot(p2_tiles[l2k], i), Add)
                    if k < nchunks:
                        nc.vector.scalar_tensor_tensor(chunk_slot(w_tiles[k], i), zview(z1),
                                                       0.0, zview(u1_sb), Max, Mult)
                    if l2k >= 0:
                        nc.vector.scalar_tensor_tensor(zview(v2), zview(z2),
                                                       0.0, zview(u2_sb), Max, Mult)

            hT = st_pool.tile([128, 128], f32r, name="hT")
            nc.scalar.activation(hT[:], z2[:], Relu, bias=0.0, scale=1.0)
            fin = psf.tile([1, BL], f32, name="fin")
            for g_hi in range(4):
                nc.tensor.matmul(fin[:], wf_sb[:, g_hi:g_hi + 1],
                                 hT[:, g_hi * BL:(g_hi + 1) * BL],
                                 start=(g_hi == 0), stop=(g_hi == 3))
            out_sb = st_pool.tile([1, BL], f32, name="out_sb")
            nc.scalar.activation(out_sb[:], fin[:], Ident, bias=0.0, scale=1.0)
            nc.gpsimd.dma_start(out_d[:], out_sb[:])

    nc.compile()
    return nc


def _prep_inputs(x, W1, u1, b1, W2, u2, b2, Wf, bf, T):
    f = np.float32
    u1c = np.where(np.abs(u1) < 1e-6, np.where(u1 >= 0, 1e-6, -1e-6), u1).astype(f)
    W2p = (W2 / u1c[None, :]).astype(f)

    nch = T // TC
    w1_rep = np.ascontiguousarray(W1.T).astype(f)   # [2, 512]
    b1_col = np.ascontiguousarray(b1.reshape(4, 128).T).astype(f)
    u1_tile = np.ascontiguousarray(
        np.broadcast_to(u1c.reshape(4, 128).T[:, :, None], (128, 4, BL)).reshape(128, 128))
    w2t = np.empty((128, 2048), f)
    for c_hi in range(4):
        for g_hi in range(4):
            blk = W2p[g_hi * 128:(g_hi + 1) * 128, c_hi * 128:(c_hi + 1) * 128]
            w2t[:, (c_hi * 4 + g_hi) * 128:(c_hi * 4 + g_hi + 1) * 128] = blk.T
    b2_col = np.ascontiguousarray(b2.reshape(4, 128).T).astype(f)
    u2_tile = np.ascontiguousarray(
        np.broadcast_to(u2.astype(f).reshape(4, 128).T[:, :, None], (128, 4, BL)).reshape(128, 128))
    wf_col = np.ascontiguousarray(Wf[0].reshape(4, 128).T).astype(f)

    in_maps = []
    for core in range(NCORES):
        xs = x[core * BL:(core + 1) * BL, :T, :]
        x_sb = np.ascontiguousarray(
            xs.reshape(BL, nch, TC, 2).transpose(1, 3, 2, 0).reshape(2 * nch, TC * BL)
        ).astype(f)  # row 2k+d, col t_lo*BL+b
        in_maps.append({
            "x_sb": x_sb, "w1_rep": w1_rep, "b1_col": b1_col, "u1_tile": u1_tile,
            "w2t": w2t, "b2_col": b2_col, "u2_tile": u2_tile, "wf_col": wf_col,
        })
    return in_maps


def kernel(x, W1, u1, b1, W2, u2, b2, Wf, bf, _T=None, _trace=False):
    x = np.asarray(x, np.float32)
    W1 = np.asarray(W1, np.float32); u1 = np.asarray(u1, np.float32)
    b1 = np.asarray(b1, np.float32); W2 = np.asarray(W2, np.float32)
    u2 = np.asarray(u2, np.float32); b2 = np.asarray(b2, np.float32)
    Wf = np.asarray(Wf, np.float32); bf = np.asarray(bf, np.float32)
    T = _T or x.shape[1]

    from concourse.bass_utils import run_bass_kernel_spmd

    if T not in _COMPILED:
        _COMPILED[T] = _build(T)
    nc = _COMPILED[T]

    in_maps = _prep_inputs(x, W1, u1, b1, W2, u2, b2, Wf, bf, T)
    res = run_bass_kernel_spmd(nc, in_maps, core_ids=list(range(NCORES)), trace=_trace)
    out = np.concatenate([res.results[i]["out"][0] for i in range(NCORES)]) + bf[0]
    kernel.last_exec_time_ns = res.exec_time_ns
    return out.astype(np.float32)



# revision 17
# speedup vs baseline: 1.6408x; 1.6408x over previous
"""Trainium2 Bass kernel for a 2-layer IndRNN (adding-problem head).

Computation (matches the reference):
    pre1 = x @ W1.T + b1                    # [B,T,H], D=2
    h1   = scan over t: h = relu(pre1_t + u1*h)   (all steps kept)
    pre2 = h1 @ W2.T + b2                   # [B,T,H]
    h2T  = scan over t: h = relu(pre2_t + u2*h)   (last step only)
    out  = h2T @ Wf.T + bf                  # [B]

Sharding: data-parallel over batch across 8 NeuronCores (32 batch each).

The scan runs as ONE custom-DVE instruction per timestep covering BOTH
layers.  State w = u*relu(z) is stored as (hi, lo) fp16 pairs (~20-bit
precision); the recurrence weight u is delivered as (u_hi, u_dlt) fp16
pairs consumed once per (layer, c_hi) sub-dimension into a persistent
per-lane register (CURR_ALU_OUT retention, as in TENSOR_PAGED_MASK).
Per element the op computes, in fp32 datapath precision:

    m  = relu(w_hi + w_lo + p) * u_reg
    hi = bitand(m, 0xFFFF0000); lo = m - hi     ->  (hi, lo) fp16 pair

Streams per 2x cycle: port0 = one state pair, port1 = one (p, junk)
pair; at each of the 8 (layer, c_hi) block boundaries port1 instead
delivers that block's (u_hi, u_dlt).

TensorE computes pre1 (K=2) and pre2 in fp16 reading the state hi
halves (stride 2); ScalarE drains 2-bank PSUM tiles into the p slots
of 4 persistent ring "pu" tiles whose u/junk positions are DMA-
initialized once.  The host folds 1/u1 into W2.
"""

import os
import sys

for _p in ("/opt/trn_rl_repo", "/root/.axon_site", "/root/.axon_site/_ro/trn_rl_repo",
           "/root/.axon_site/_ro/pypackages"):
    if os.path.isdir(_p) and _p not in sys.path:
        sys.path.append(_p)

import numpy as np

B, T_FULL, D, H = 256, 2048, 2, 512
NCORES = 8
BL = B // NCORES          # 32 batch per core
TC = 16                   # timesteps per chunk
LAG = 2                   # layer-2 chunk lag behind layer 1

BLK = 66                  # pu halves per (l, c, t): [u_hi, u_dlt, 32 x (p, junk)]
SLOT = 64                 # state halves per (l, c, t): 32 x (hi, lo)

_COMPILED = {}
_OP = {}


def _register_op():
    """Register the fused IndRNN-step custom DVE op (hand-written 2x uops)."""
    if "INDRNN_STEP_ANT" in _OP:
        return _OP["INDRNN_STEP_ANT"]
    from concourse import dve_ops
    from concourse.dve_spec import Spec, Src0, Src1, relu as sp_relu
    from concourse.dve_uop import (
        UopConfig, DveOpSpec, InpSel, OutSel, OutPath, AluOp, AluInp,
        DelayInp, Trigger,
    )

    def _ref(in0, in1, s0, s1, imm2):
        # in0: state pairs [P, S, 64]; in1: [P, S, 66]; out like in0.
        a0 = np.asarray(in0, np.float32)
        a1 = np.asarray(in1, np.float32)
        w = a0[..., 0::2] + a0[..., 1::2]              # [P, S, 32]
        u = (a1[..., 0] + a1[..., 1])[..., None]       # [P, S, 1]
        p = a1[..., 2::2]                              # [P, S, 32]
        m = np.maximum(w + p, 0.0) * u
        m32 = m.astype(np.float32)
        hi = (m32.view(np.int32) & np.int32(-65536)).view(np.float32)
        lo = m32 - hi
        out = np.empty_like(a0)
        out[..., 0::2] = hi
        out[..., 1::2] = lo
        return out

    spec = Spec(body=sp_relu(Src0) * Src1, reference=_ref)  # body nominal only

    P = DelayInp.PREV_DELAY

    def steady():
        u = UopConfig()
        u.enable_input(InpSel.SRC_0, 0)        # w_hi
        u.enable_input(InpSel.SRC_1, 1)        # p
        u.enable_input(InpSel.SRC_0_HI, 2)     # w_lo
        u.enable_input(InpSel.SRC_1_HI, 3)     # junk
        u.enable_input(InpSel.ZERO, 4)
        u.enable_input(InpSel.MASK16_SL16, 5)
        u.require_inp0 = 1
        u.require_inp1 = 1
        dp = u.datapath_config
        # b0: w = w_hi + w_lo ; carry p, zero, mask
        dp[0].enable_alu(AluOp.ADD, AluInp.PREV_ALU_OUT, AluInp.PREV_DELAY_1)
        dp[0].enable_delay_from_src(P, 0)      # p      <- lane1
        dp[0].enable_delay_from_src(P, 3)      # zero   <- lane4
        dp[0].enable_delay_from_src(P, 4)      # mask   <- lane5
        # b1: z = w + p
        dp[1].enable_alu(AluOp.ADD, AluInp.PREV_ALU_OUT, AluInp.PREV_DELAY_0)
        dp[1].pass_through_delay(3, 4)
        # b2: r = max(z, 0)
        dp[2].enable_alu(AluOp.MAX, AluInp.PREV_ALU_OUT, AluInp.PREV_DELAY_3)
        dp[2].pass_through_delay(4)
        # b3: u-register hold (CURR_ALU_OUT feedback); carry r in d0
        dp[3].enable_alu(AluOp.BYPASS, AluInp.CURR_ALU_OUT, AluInp.CURR_ALU_OUT)
        dp[3].enable_delay_from_src(DelayInp.PREV_ALU_OUT, 0)   # r
        dp[3].pass_through_delay(4)
        # b4: m = u_reg * r
        dp[4].enable_alu(AluOp.MULTIPLY, AluInp.PREV_ALU_OUT, AluInp.PREV_DELAY_0)
        dp[4].pass_through_delay(4)
        # b5: hi = m & 0xFFFF0000 ; carry m in d1
        dp[5].enable_alu(AluOp.BITWISE_AND, AluInp.PREV_ALU_OUT, AluInp.PREV_DELAY_4)
        dp[5].enable_delay_from_src(DelayInp.PREV_ALU_OUT, 1)   # m
        # b6: lo = m - hi ; carry hi in d0
        dp[6].enable_alu(AluOp.SUBTRACT, AluInp.PREV_DELAY_1, AluInp.PREV_ALU_OUT)
        dp[6].enable_delay_from_src(DelayInp.PREV_ALU_OUT, 0)   # hi
        # b7: pass lo through ALU; hi rides d0
        dp[7].pass_through_alu()
        dp[7].pass_through_delay(0)
        # engine convention (measured): WR0_LO -> even half, WR0_HI -> odd.
        # hi must land at evens (reference + matmul readout read evens).
        u.enable_output(OutSel.DELAY_0, OutPath.WR0_LO)    # hi -> even
        u.enable_output(OutSel.ALU_OUT, OutPath.WR0_HI)    # lo -> odd
        return u

    def boundary():
        # consume one (u_hi, u_dlt) pair from port1; load u_reg into b3 flop
        u = UopConfig()
        u.enable_input(InpSel.SRC_1, 1)
        u.enable_input(InpSel.SRC_1_HI, 3)
        u.require_inp0 = 0
        u.require_inp1 = 1
        u.repeat_count = 1
        dp = u.datapath_config
        # b0: u32 = u_hi + u_dlt
        dp[0].enable_alu(AluOp.ADD, AluInp.PREV_DELAY_0, AluInp.PREV_DELAY_2)
        dp[1].pass_through_alu()
        dp[2].pass_through_alu()
        dp[3].pass_through_alu()   # lands u32 in b3's out flop
        return u

    u0 = boundary()               # entry: load block-0's u
    u0.trigger = (Trigger.COUNT, Trigger.NONE, Trigger.NONE)
    u0.next_uop = (1, 0, 0)
    u1 = steady()                 # steady: one logical element per cycle
    u1.trigger = (Trigger.SRC_TENSOR_DONE, Trigger.SUB_DIM_DONE, Trigger.NONE)
    u1.next_uop = (0, 2, 0)
    u2 = boundary()               # subdim boundary: reload u
    u2.trigger = (Trigger.SRC_TENSOR_DONE, Trigger.COUNT, Trigger.NONE)
    u2.next_uop = (0, 1, 0)

    uops = [u0, u1, u2]

    row = 1 + len(dve_ops.OPS)
    name = "INDRNN_STEP_ANT"

    built = DveOpSpec(name=name, uops=uops, uops_2x=uops,
                      opcode=row, perf_max=1, rd1_en=True)

    class _HandOp(dve_ops.DveOp):
        def compile(self, ver):
            assert ver == "v3", f"hand-built op only supports v3, got {ver}"
            return built

    op = _HandOp(name=name, spec=spec, subdim=True, uops_sha={})
    dve_ops.OPS.append(op)
    dve_ops.CUSTOM_DVE_SPECS[name] = spec
    dve_ops._SUB_OPCODE_FOR_NAME[name] = row
    _OP[name] = op
    return op


def _emit_step(nc, op, out, in0, in1):
    """Emit the fused step instruction with perf_max=1 (2x mode reachable)."""
    from concourse import bass_isa, mybir

    v = nc.vector
    if op.name not in nc.m.ant_custom_dve_ops:
        nc.m.ant_custom_dve_ops = sorted({*nc.m.ant_custom_dve_ops, op.name})
    shape = bass_isa.CustomDveShape.STT          # 2-free-dim src1
    isa_opcode = nc.isa.Opcode[
        f"NEURON_ISA_TPB_OPCODE_CUSTOM_DVE_ANT_{shape.slot()}"
    ].value
    imm = mybir.ImmediateValue(dtype=mybir.dt.float32, value=0.0)
    ins = [v.lower_ap(in0, for_isa=True, opt=False),
           v.lower_ap(in1, for_isa=True, opt=False),
           imm,
           mybir.ImmediateValue(dtype=mybir.dt.float32, value=0.0)]
    outs = [v.lower_ap(out, for_isa=True, opt=False)]
    from concourse.dve_ops import get_dve_sub_opcode
    return v.add_instruction(bass_isa.InstCustomDveAnt(
        name=v.bass.get_next_instruction_name(),
        op_name=op.name,
        rd1_en=True,
        subdim=0x02,
        imm2=0.0,
        shape=shape,
        row=get_dve_sub_opcode(op.name),
        perf_max=1,
        isa_opcode=isa_opcode,
        ins=ins,
        outs=outs,
    ))


def _build(T, with_b2=True):
    import contextlib
    from concourse import tile, bacc, mybir

    op = _register_op()

    nchunks = T // TC
    nk = nchunks + LAG
    NPU = 4                    # pu ring depth

    f16 = mybir.dt.float16
    f32 = mybir.dt.float32
    f32r = mybir.dt.float32r
    Add = mybir.AluOpType.add
    Mult = mybir.AluOpType.mult
    Ident = mybir.ActivationFunctionType.Identity

    CB = TC * BL               # elems per (l, c_hi) per chunk = 512
    PU_F = 2 * 4 * TC * BLK    # pu tile halves = 8448
    ST_F = 2 * 4 * TC * SLOT   # state tile halves = 8192

    nc = bacc.Bacc("TRN2", target_bir_lowering=False, debug=False)

    x_d = nc.dram_tensor("x_sb", [3 * nchunks, CB], f16, kind="ExternalInput").ap()
    w1_d = nc.dram_tensor("w1_rep", [3, 512], f16, kind="ExternalInput").ap()
    w2_d = nc.dram_tensor("w2t", [128, 2048], f16, kind="ExternalInput").ap()
    b2_d = nc.dram_tensor("b2_row", [1, 512], f16, kind="ExternalInput").ap()
    pu_d = nc.dram_tensor("pu_init", [128, PU_F], f16, kind="ExternalInput").ap()
    iu2_d = nc.dram_tensor("inv_u2", [128, 128], f32, kind="ExternalInput").ap()
    wf_d = nc.dram_tensor("wf_col", [128, 4], f32r, kind="ExternalInput").ap()
    out_d = nc.dram_tensor("out", [1, BL], f32, kind="ExternalOutput").ap()

    with tile.TileContext(nc) as tc:
        with contextlib.ExitStack() as ctx:
            consts = ctx.enter_context(tc.tile_pool(name="consts", bufs=1))
            st_pool = ctx.enter_context(tc.tile_pool(name="st", bufs=3))
            xs_pool = ctx.enter_context(tc.tile_pool(name="xs", bufs=4))
            misc = ctx.enter_context(tc.tile_pool(name="misc", bufs=1))
            ps1 = ctx.enter_context(tc.tile_pool(name="ps1", bufs=2, space="PSUM"))
            ps2 = ctx.enter_context(tc.tile_pool(name="ps2", bufs=2, space="PSUM"))

            w1_sb = consts.tile([3, 512], f16, name="w1_sbt")
            w2_sb = consts.tile([128, 2048], f16, name="w2_sbt")
            b2_sb = consts.tile([1, 512], f16, name="b2_sbt")
            iu2_sb = consts.tile([128, 128], f32, name="iu2_sbt")
            wf_sb = consts.tile([128, 4], f32r, name="wf_sbt")
            ones_sb = consts.tile([1, CB], f16, name="ones_sbt")
            pu = [consts.tile([128, PU_F], f16, name=f"pu{m}") for m in range(NPU)]

            nc.gpsimd.dma_start(w1_sb[:], w1_d[:])
            nc.vector.memset(ones_sb[:], 1.0)
            for m in range(NPU):
                nc.gpsimd.dma_start(pu[m][:], pu_d[:])

            zst = misc.tile([128, 4 * SLOT], f16, name="zst")   # zero state pairs
            nc.vector.memset(zst[:], 0.0)

            st_tiles = {}

            def st_slot(k, i, lsel=None):
                # state AP [128, S, 64] at step i (lsel: 0/1 for one layer)
                v = st_tiles[k][:].rearrange(
                    "p (l c t s) -> p (l c) t s", l=2, c=4, t=TC, s=SLOT)
                if lsel is None:
                    return v[:, :, i, :]
                return v[:, 4 * lsel:4 * lsel + 4, i, :]

            def pu_slot(k, i, lsel=None):
                # pu AP [128, S, 66] at step i
                v = pu[k % NPU][:].rearrange(
                    "p (l c t s) -> p (l c) t s", l=2, c=4, t=TC, s=BLK)
                if lsel is None:
                    return v[:, :, i, :]
                return v[:, 4 * lsel:4 * lsel + 4, i, :]

            def p_drain_ap(k, lsel, cpair):
                # drain target: p positions of pu tile for (l, c in {2*cpair, +1})
                # dims: (c:2, t:TC, b:32) ; halves offset 2 + 2b
                v = pu[k % NPU][:].rearrange(
                    "p (l c t s) -> p l c t s", l=2, c=4, t=TC, s=BLK)
                return v[:, lsel, 2 * cpair:2 * cpair + 2, :, 2::2]

            def mm_rhs(k, c):
                # matmul rhs: hi halves of layer-1 state, block c: [128, t, b]
                v = st_tiles[k][:].rearrange(
                    "p (l c t b two) -> p l c t b two", l=2, c=4, t=TC, b=BL, two=2)
                return v[:, 0, c, :, :, 0]

            def p1_matmul(k):
                xst = xs_pool.tile([3, CB], f16, name=f"xst_{k}", tag="xst")
                nc.gpsimd.dma_start(xst[:], x_d[3 * k:3 * k + 3, :])
                for cpair in range(2):
                    ps = ps1.tile([128, 2 * CB], f32, name=f"p1ps_{k}_{cpair}", tag="p1ps")
                    for ci in range(2):
                        c_hi = 2 * cpair + ci
                        lhsT = w1_sb[0:3, c_hi * 128:(c_hi + 1) * 128]
                        nc.tensor.matmul(ps[:, ci * CB:(ci + 1) * CB], lhsT, xst[:],
                                         start=True, stop=True)
                    nc.scalar.activation(
                        p_drain_ap(k, 0, cpair), ps[:].rearrange(
                            "p (c t b) -> p c t b", c=2, t=TC, b=BL),
                        Ident, bias=0.0, scale=1.0)

            def w2_matmul(j):
                # pre2 of chunk j -> pu tile of chunk j+LAG, l=1
                for gpair in range(2):
                    ps = ps2.tile([128, 2 * CB], f32, name=f"p2ps_{j}_{gpair}", tag="p2ps")
                    for gi in range(2):
                        g = 2 * gpair + gi
                        for c in range(4):
                            lhsT = w2_sb[:, (c * 4 + g) * 128:(c * 4 + g + 1) * 128]
                            nc.tensor.matmul(ps[:, gi * CB:(gi + 1) * CB], lhsT,
                                             mm_rhs(j, c),
                                             start=(c == 0),
                                             stop=(c == 3 and not with_b2))
                        if with_b2:
                            nc.tensor.matmul(ps[:, gi * CB:(gi + 1) * CB],
                                             b2_sb[0:1, g * 128:(g + 1) * 128],
                                             ones_sb[0:1, :],
                                             start=False, stop=True)
                    nc.scalar.activation(
                        p_drain_ap(j + LAG, 1, gpair), ps[:].rearrange(
                            "p (c t b) -> p c t b", c=2, t=TC, b=BL),
                        Ident, bias=0.0, scale=1.0)

            p1_matmul(0)
            for sb, dr in ((b2_sb, b2_d), (iu2_sb, iu2_d), (wf_sb, wf_d),
                           (w2_sb, w2_d)):
                nc.gpsimd.dma_start(sb[:], dr[:])
            if nchunks > 1:
                p1_matmul(1)

            zview = zst[:].rearrange("p (c s) -> p c s", c=4, s=SLOT)

            for k in range(nk):
                if 1 <= k <= nchunks:
                    w2_matmul(k - 1)
                if k + 2 < nchunks:
                    p1_matmul(k + 2)

                l2_active = k >= LAG
                l1_active = k < nchunks
                st_tiles[k] = st_pool.tile([128, ST_F], f16, name=f"st_{k}", tag="st")

                for i in range(TC):
                    if l1_active and l2_active:
                        if i == 0 and k == LAG:
                            _emit_step(nc, op, st_slot(k, 0, 0),
                                       st_slot(k - 1, TC - 1, 0), pu_slot(k, 0, 0))
                            _emit_step(nc, op, st_slot(k, 0, 1),
                                       zview, pu_slot(k, 0, 1))
                        else:
                            prev = (st_slot(k - 1, TC - 1) if i == 0
                                    else st_slot(k, i - 1))
                            _emit_step(nc, op, st_slot(k, i), prev, pu_slot(k, i))
                    elif l1_active:
                        if i == 0 and k == 0:
                            prev = zview
                        elif i == 0:
                            prev = st_slot(k - 1, TC - 1, 0)
                        else:
                            prev = st_slot(k, i - 1, 0)
                        _emit_step(nc, op, st_slot(k, i, 0), prev, pu_slot(k, i, 0))
                    else:
                        if i == 0:
                            prev = st_slot(k - 1, TC - 1, 1)
                        else:
                            prev = st_slot(k, i - 1, 1)
                        _emit_step(nc, op, st_slot(k, i, 1), prev, pu_slot(k, i, 1))

            # final: w2 = hi + lo of last layer-2 state; h2T = w2 * (1/u2)
            last = st_tiles[nk - 1][:].rearrange(
                "p (l c t b two) -> p l c t b two", l=2, c=4, t=TC, b=BL, two=2)
            w2f = misc.tile([128, 128], f32, name="w2f")
            nc.vector.tensor_tensor(
                w2f[:].rearrange("p (c b) -> p c b", c=4, b=BL),
                last[:, 1, :, TC - 1, :, 0], last[:, 1, :, TC - 1, :, 1], Add)
            hT = misc.tile([128, 128], f32r, name="hT")
            nc.vector.tensor_tensor(hT[:], w2f[:], iu2_sb[:], Mult)
            finps = ps2.tile([128, 2 * CB], f32, name="finps", tag="p2ps")
            fin = finps[0:1, 0:BL]
            for g_hi in range(4):
                nc.tensor.matmul(fin, wf_sb[:, g_hi:g_hi + 1],
                                 hT[:, g_hi * BL:(g_hi + 1) * BL],
                                 start=(g_hi == 0), stop=(g_hi == 3))
            out_sb = misc.tile([1, BL], f32, name="out_sb")
            nc.scalar.activation(out_sb[:], fin, Ident, bias=0.0, scale=1.0)
            nc.gpsimd.dma_start(out_d[:], out_sb[:])

    nc.compile()
    return nc


def _prep_inputs(x, W1, u1, b1, W2, u2, b2, Wf, bf, T):
    f = np.float32
    u1c = np.where(np.abs(u1) < 1e-6, np.where(u1 >= 0, 1e-6, -1e-6), u1).astype(f)
    u2c = np.where(np.abs(u2) < 1e-6, np.where(u2 >= 0, 1e-6, -1e-6), u2).astype(f)
    # compensate the mean of the hi-half truncation (hi = trunc_bf16(w)) seen
    # by the pre2 matmul: E[w - hi] ~ 2^-9 |w|
    W2p = ((W2 / u1c[None, :]) * (1.0 + 2.0 ** -9)).astype(f)

    nch = T // TC
    w1_rep = np.concatenate([W1.T, b1[None, :]], 0).astype(np.float16)  # [3, 512]
    b2_row = b2[None, :].astype(np.float16)                             # [1, 512]
    w2t = np.empty((128, 2048), np.float16)
    for c_hi in range(4):
        for g_hi in range(4):
            blk = W2p[g_hi * 128:(g_hi + 1) * 128, c_hi * 128:(c_hi + 1) * 128]
            w2t[:, (c_hi * 4 + g_hi) * 128:(c_hi * 4 + g_hi + 1) * 128] = blk.T
    wf_col = np.ascontiguousarray(Wf[0].reshape(4, 128).T).astype(f)
    iu2 = np.ascontiguousarray(
        np.broadcast_to((1.0 / u2c).reshape(4, 128).T[:, :, None],
                        (128, 4, BL)).reshape(128, 128)).astype(f)

    # pu init pattern [128, (l, c, t, 66)]: [u_hi, u_dlt, 32 x (p=0, junk=0)]
    pu = np.zeros((128, 2, 4, TC, BLK), np.float16)
    for lsel, uv in ((0, u1c), (1, u2c)):
        ucol = uv.reshape(4, 128).T                       # [c_lo, c_hi]
        uhi = ucol.astype(np.float16)
        udl = (ucol - uhi.astype(f)).astype(np.float16)
        pu[:, lsel, :, :, 0] = uhi[:, :, None]
        pu[:, lsel, :, :, 1] = udl[:, :, None]
    pu_init = np.ascontiguousarray(pu.reshape(128, 2 * 4 * TC * BLK))

    in_maps = []
    for core in range(NCORES):
        xs = x[core * BL:(core + 1) * BL, :T, :]
        x2 = xs.reshape(BL, nch, TC, 2).transpose(1, 3, 2, 0)     # [nch, 2, TC, BL]
        x_sb = np.empty((nch, 3, TC * BL), np.float16)
        x_sb[:, 0:2] = x2.reshape(nch, 2, TC * BL)
        x_sb[:, 2] = 1.0
        x_sb = np.ascontiguousarray(x_sb.reshape(3 * nch, TC * BL))
        in_maps.append({
            "x_sb": x_sb, "w1_rep": w1_rep, "pu_init": pu_init,
            "w2t": w2t, "b2_row": b2_row, "inv_u2": iu2, "wf_col": wf_col,
        })
    return in_maps


def kernel(x, W1, u1, b1, W2, u2, b2, Wf, bf, _T=None, _trace=False):
    x = np.asarray(x, np.float32)
    W1 = np.asarray(W1, np.float32); u1 = np.asarray(u1, np.float32)
    b1 = np.asarray(b1, np.float32); W2 = np.asarray(W2, np.float32)
    u2 = np.asarray(u2, np.float32); b2 = np.asarray(b2, np.float32)
    Wf = np.asarray(Wf, np.float32); bf = np.asarray(bf, np.float32)
    T = _T or x.shape[1]

    from concourse.bass_utils import run_bass_kernel_spmd

    with_b2 = bool(np.any(b2))
    key = (T, with_b2)
    if key not in _COMPILED:
        _COMPILED[key] = _build(T, with_b2=with_b2)
    nc = _COMPILED[key]

    in_maps = _prep_inputs(x, W1, u1, b1, W2, u2, b2, Wf, bf, T)
    res = run_bass_kernel_spmd(nc, in_maps, core_ids=list(range(NCORES)), trace=_trace)
    out = np.concatenate([res.results[i]["out"][0] for i in range(NCORES)]) + bf[0]
    kernel.last_exec_time_ns = res.exec_time_ns
    return out.astype(np.float32)


# revision 18
# speedup vs baseline: 1.6471x; 1.0038x over previous
"""Trainium2 Bass kernel for a 2-layer IndRNN (adding-problem head).

Computation (matches the reference):
    pre1 = x @ W1.T + b1                    # [B,T,H], D=2
    h1   = scan over t: h = relu(pre1_t + u1*h)   (all steps kept)
    pre2 = h1 @ W2.T + b2                   # [B,T,H]
    h2T  = scan over t: h = relu(pre2_t + u2*h)   (last step only)
    out  = h2T @ Wf.T + bf                  # [B]

Sharding: data-parallel over batch across 8 NeuronCores (32 batch each).

The scan runs as ONE custom-DVE instruction per timestep covering BOTH
layers.  State w = u*relu(z) is stored as (hi, lo) fp16 pairs (~20-bit
precision); the recurrence weight u is delivered as (u_hi, u_dlt) fp16
pairs consumed once per (layer, c_hi) sub-dimension into a persistent
per-lane register (CURR_ALU_OUT retention, as in TENSOR_PAGED_MASK).
Per element the op computes, in fp32 datapath precision:

    m  = relu(w_hi + w_lo + p) * u_reg
    hi = bitand(m, 0xFFFF0000); lo = m - hi     ->  (hi, lo) fp16 pair

Streams per 2x cycle: port0 = one state pair, port1 = one (p, junk)
pair; at each of the 8 (layer, c_hi) block boundaries port1 instead
delivers that block's (u_hi, u_dlt).

TensorE computes pre1 (K=2) and pre2 in fp16 reading the state hi
halves (stride 2); ScalarE drains 2-bank PSUM tiles into the p slots
of 4 persistent ring "pu" tiles whose u/junk positions are DMA-
initialized once.  The host folds 1/u1 into W2.
"""

import os
import sys

for _p in ("/opt/trn_rl_repo", "/root/.axon_site", "/root/.axon_site/_ro/trn_rl_repo",
           "/root/.axon_site/_ro/pypackages"):
    if os.path.isdir(_p) and _p not in sys.path:
        sys.path.append(_p)

import numpy as np

B, T_FULL, D, H = 256, 2048, 2, 512
NCORES = 8
BL = B // NCORES          # 32 batch per core
TC = 16                   # timesteps per chunk
LAG = 3                   # layer-2 chunk lag behind layer 1

BLK = 66                  # pu halves per (l, c, t): [u_hi, u_dlt, 32 x (p, junk)]
SLOT = 64                 # state halves per (l, c, t): 32 x (hi, lo)

_COMPILED = {}
_OP = {}


def _register_op():
    """Register the fused IndRNN-step custom DVE op (hand-written 2x uops)."""
    if "INDRNN_STEP_ANT" in _OP:
        return _OP["INDRNN_STEP_ANT"]
    from concourse import dve_ops
    from concourse.dve_spec import Spec, Src0, Src1, relu as sp_relu
    from concourse.dve_uop import (
        UopConfig, DveOpSpec, InpSel, OutSel, OutPath, AluOp, AluInp,
        DelayInp, Trigger,
    )

    def _ref(in0, in1, s0, s1, imm2):
        # in0: state pairs [P, S, 64]; in1: [P, S, 66]; out like in0.
        a0 = np.asarray(in0, np.float32)
        a1 = np.asarray(in1, np.float32)
        w = a0[..., 0::2] + a0[..., 1::2]              # [P, S, 32]
        u = (a1[..., 0] + a1[..., 1])[..., None]       # [P, S, 1]
        p = a1[..., 2::2]                              # [P, S, 32]
        m = np.maximum(w + p, 0.0) * u
        m32 = m.astype(np.float32)
        hi = (m32.view(np.int32) & np.int32(-65536)).view(np.float32)
        lo = m32 - hi
        out = np.empty_like(a0)
        out[..., 0::2] = hi
        out[..., 1::2] = lo
        return out

    spec = Spec(body=sp_relu(Src0) * Src1, reference=_ref)  # body nominal only

    P = DelayInp.PREV_DELAY

    def steady():
        u = UopConfig()
        u.enable_input(InpSel.SRC_0, 0)        # w_hi
        u.enable_input(InpSel.SRC_1, 1)        # p
        u.enable_input(InpSel.SRC_0_HI, 2)     # w_lo
        u.enable_input(InpSel.SRC_1_HI, 3)     # junk
        u.enable_input(InpSel.ZERO, 4)
        u.enable_input(InpSel.MASK16_SL16, 5)
        u.require_inp0 = 1
        u.require_inp1 = 1
        dp = u.datapath_config
        # b0: w = w_hi + w_lo ; carry p, zero, mask
        dp[0].enable_alu(AluOp.ADD, AluInp.PREV_ALU_OUT, AluInp.PREV_DELAY_1)
        dp[0].enable_delay_from_src(P, 0)      # p      <- lane1
        dp[0].enable_delay_from_src(P, 3)      # zero   <- lane4
        dp[0].enable_delay_from_src(P, 4)      # mask   <- lane5
        # b1: z = w + p
        dp[1].enable_alu(AluOp.ADD, AluInp.PREV_ALU_OUT, AluInp.PREV_DELAY_0)
        dp[1].pass_through_delay(3, 4)
        # b2: r = max(z, 0)
        dp[2].enable_alu(AluOp.MAX, AluInp.PREV_ALU_OUT, AluInp.PREV_DELAY_3)
        dp[2].pass_through_delay(4)
        # b3: u-register hold (CURR_ALU_OUT feedback); carry r in d0
        dp[3].enable_alu(AluOp.BYPASS, AluInp.CURR_ALU_OUT, AluInp.CURR_ALU_OUT)
        dp[3].enable_delay_from_src(DelayInp.PREV_ALU_OUT, 0)   # r
        dp[3].pass_through_delay(4)
        # b4: m = u_reg * r
        dp[4].enable_alu(AluOp.MULTIPLY, AluInp.PREV_ALU_OUT, AluInp.PREV_DELAY_0)
        dp[4].pass_through_delay(4)
        # b5: hi = m & 0xFFFF0000 ; carry m in d1
        dp[5].enable_alu(AluOp.BITWISE_AND, AluInp.PREV_ALU_OUT, AluInp.PREV_DELAY_4)
        dp[5].enable_delay_from_src(DelayInp.PREV_ALU_OUT, 1)   # m
        # b6: lo = m - hi ; carry hi in d0
        dp[6].enable_alu(AluOp.SUBTRACT, AluInp.PREV_DELAY_1, AluInp.PREV_ALU_OUT)
        dp[6].enable_delay_from_src(DelayInp.PREV_ALU_OUT, 0)   # hi
        # b7: pass lo through ALU; hi rides d0
        dp[7].pass_through_alu()
        dp[7].pass_through_delay(0)
        # engine convention (measured): WR0_LO -> even half, WR0_HI -> odd.
        # hi must land at evens (reference + matmul readout read evens).
        u.enable_output(OutSel.DELAY_0, OutPath.WR0_LO)    # hi -> even
        u.enable_output(OutSel.ALU_OUT, OutPath.WR0_HI)    # lo -> odd
        return u

    def boundary():
        # consume one (u_hi, u_dlt) pair from port1; load u_reg into b3 flop
        u = UopConfig()
        u.enable_input(InpSel.SRC_1, 1)
        u.enable_input(InpSel.SRC_1_HI, 3)
        u.require_inp0 = 0
        u.require_inp1 = 1
        u.repeat_count = 1
        dp = u.datapath_config
        # b0: u32 = u_hi + u_dlt
        dp[0].enable_alu(AluOp.ADD, AluInp.PREV_DELAY_0, AluInp.PREV_DELAY_2)
        dp[1].pass_through_alu()
        dp[2].pass_through_alu()
        dp[3].pass_through_alu()   # lands u32 in b3's out flop
        return u

    u0 = boundary()               # entry: load block-0's u
    u0.trigger = (Trigger.COUNT, Trigger.NONE, Trigger.NONE)
    u0.next_uop = (1, 0, 0)
    u1 = steady()                 # steady: one logical element per cycle
    u1.trigger = (Trigger.SRC_TENSOR_DONE, Trigger.SUB_DIM_DONE, Trigger.NONE)
    u1.next_uop = (0, 2, 0)
    u2 = boundary()               # subdim boundary: reload u
    u2.trigger = (Trigger.SRC_TENSOR_DONE, Trigger.COUNT, Trigger.NONE)
    u2.next_uop = (0, 1, 0)

    uops = [u0, u1, u2]

    row = 1 + len(dve_ops.OPS)
    name = "INDRNN_STEP_ANT"

    built = DveOpSpec(name=name, uops=uops, uops_2x=uops,
                      opcode=row, perf_max=1, rd1_en=True)

    class _HandOp(dve_ops.DveOp):
        def compile(self, ver):
            assert ver == "v3", f"hand-built op only supports v3, got {ver}"
            return built

    op = _HandOp(name=name, spec=spec, subdim=True, uops_sha={})
    dve_ops.OPS.append(op)
    dve_ops.CUSTOM_DVE_SPECS[name] = spec
    dve_ops._SUB_OPCODE_FOR_NAME[name] = row
    _OP[name] = op
    return op


def _emit_step(nc, op, out, in0, in1):
    """Emit the fused step instruction with perf_max=1 (2x mode reachable)."""
    from concourse import bass_isa, mybir

    v = nc.vector
    if op.name not in nc.m.ant_custom_dve_ops:
        nc.m.ant_custom_dve_ops = sorted({*nc.m.ant_custom_dve_ops, op.name})
    shape = bass_isa.CustomDveShape.STT          # 2-free-dim src1
    isa_opcode = nc.isa.Opcode[
        f"NEURON_ISA_TPB_OPCODE_CUSTOM_DVE_ANT_{shape.slot()}"
    ].value
    imm = mybir.ImmediateValue(dtype=mybir.dt.float32, value=0.0)
    ins = [v.lower_ap(in0, for_isa=True, opt=False),
           v.lower_ap(in1, for_isa=True, opt=False),
           imm,
           mybir.ImmediateValue(dtype=mybir.dt.float32, value=0.0)]
    outs = [v.lower_ap(out, for_isa=True, opt=False)]
    from concourse.dve_ops import get_dve_sub_opcode
    return v.add_instruction(bass_isa.InstCustomDveAnt(
        name=v.bass.get_next_instruction_name(),
        op_name=op.name,
        rd1_en=True,
        subdim=0x02,
        imm2=0.0,
        shape=shape,
        row=get_dve_sub_opcode(op.name),
        perf_max=1,
        isa_opcode=isa_opcode,
        ins=ins,
        outs=outs,
    ))


def _build(T, with_b2=True):
    import contextlib
    from concourse import tile, bacc, mybir

    op = _register_op()

    nchunks = T // TC
    nk = nchunks + LAG
    NPU = 4                    # pu ring depth

    f16 = mybir.dt.float16
    f32 = mybir.dt.float32
    f32r = mybir.dt.float32r
    Add = mybir.AluOpType.add
    Mult = mybir.AluOpType.mult
    Ident = mybir.ActivationFunctionType.Identity

    CB = TC * BL               # elems per (l, c_hi) per chunk = 512
    PU_F = 2 * 4 * TC * BLK    # pu tile halves = 8448
    ST_F = 2 * 4 * TC * SLOT   # state tile halves = 8192

    nc = bacc.Bacc("TRN2", target_bir_lowering=False, debug=False)

    x_d = nc.dram_tensor("x_sb", [3 * nchunks, CB], f16, kind="ExternalInput").ap()
    w1_d = nc.dram_tensor("w1_rep", [3, 512], f16, kind="ExternalInput").ap()
    w2_d = nc.dram_tensor("w2t", [128, 2048], f16, kind="ExternalInput").ap()
    b2_d = nc.dram_tensor("b2_row", [1, 512], f16, kind="ExternalInput").ap()
    pu_d = nc.dram_tensor("pu_init", [128, PU_F], f16, kind="ExternalInput").ap()
    iu2_d = nc.dram_tensor("inv_u2", [128, 128], f32, kind="ExternalInput").ap()
    wf_d = nc.dram_tensor("wf_col", [128, 4], f32r, kind="ExternalInput").ap()
    out_d = nc.dram_tensor("out", [1, BL], f32, kind="ExternalOutput").ap()

    with tile.TileContext(nc) as tc:
        with contextlib.ExitStack() as ctx:
            consts = ctx.enter_context(tc.tile_pool(name="consts", bufs=1))
            st_pool = ctx.enter_context(tc.tile_pool(name="st", bufs=3))
            xs_pool = ctx.enter_context(tc.tile_pool(name="xs", bufs=4))
            misc = ctx.enter_context(tc.tile_pool(name="misc", bufs=1))
            ps1 = ctx.enter_context(tc.tile_pool(name="ps1", bufs=2, space="PSUM"))
            ps2 = ctx.enter_context(tc.tile_pool(name="ps2", bufs=2, space="PSUM"))

            w1_sb = consts.tile([3, 512], f16, name="w1_sbt")
            w2_sb = consts.tile([128, 2048], f16, name="w2_sbt")
            b2_sb = consts.tile([1, 512], f16, name="b2_sbt")
            iu2_sb = consts.tile([128, 128], f32, name="iu2_sbt")
            wf_sb = consts.tile([128, 4], f32r, name="wf_sbt")
            ones_sb = consts.tile([1, CB], f16, name="ones_sbt")
            pu = [consts.tile([128, PU_F], f16, name=f"pu{m}") for m in range(NPU)]

            nc.gpsimd.dma_start(w1_sb[:], w1_d[:])
            nc.vector.memset(ones_sb[:], 1.0)
            for m in range(NPU):
                nc.gpsimd.dma_start(pu[m][:], pu_d[:])

            zst = misc.tile([128, 4 * SLOT], f16, name="zst")   # zero state pairs
            nc.vector.memset(zst[:], 0.0)

            st_tiles = {}

            def st_slot(k, i, lsel=None):
                # state AP [128, S, 64] at step i (lsel: 0/1 for one layer)
                v = st_tiles[k][:].rearrange(
                    "p (l c t s) -> p (l c) t s", l=2, c=4, t=TC, s=SLOT)
                if lsel is None:
                    return v[:, :, i, :]
                return v[:, 4 * lsel:4 * lsel + 4, i, :]

            def pu_slot(k, i, lsel=None):
                # pu AP [128, S, 66] at step i
                v = pu[k % NPU][:].rearrange(
                    "p (l c t s) -> p (l c) t s", l=2, c=4, t=TC, s=BLK)
                if lsel is None:
                    return v[:, :, i, :]
                return v[:, 4 * lsel:4 * lsel + 4, i, :]

            def p_drain_ap(k, lsel, cpair):
                # drain target: p positions of pu tile for (l, c in {2*cpair, +1})
                # dims: (c:2, t:TC, b:32) ; halves offset 2 + 2b
                v = pu[k % NPU][:].rearrange(
                    "p (l c t s) -> p l c t s", l=2, c=4, t=TC, s=BLK)
                return v[:, lsel, 2 * cpair:2 * cpair + 2, :, 2::2]

            def mm_rhs(k, c):
                # matmul rhs: hi halves of layer-1 state, block c: [128, t, b]
                v = st_tiles[k][:].rearrange(
                    "p (l c t b two) -> p l c t b two", l=2, c=4, t=TC, b=BL, two=2)
                return v[:, 0, c, :, :, 0]

            def p1_matmul(k):
                xst = xs_pool.tile([3, CB], f16, name=f"xst_{k}", tag="xst")
                nc.gpsimd.dma_start(xst[:], x_d[3 * k:3 * k + 3, :])
                for cpair in range(2):
                    ps = ps1.tile([128, 2 * CB], f32, name=f"p1ps_{k}_{cpair}", tag="p1ps")
                    for ci in range(2):
                        c_hi = 2 * cpair + ci
                        lhsT = w1_sb[0:3, c_hi * 128:(c_hi + 1) * 128]
                        nc.tensor.matmul(ps[:, ci * CB:(ci + 1) * CB], lhsT, xst[:],
                                         start=True, stop=True)
                    nc.scalar.activation(
                        p_drain_ap(k, 0, cpair), ps[:].rearrange(
                            "p (c t b) -> p c t b", c=2, t=TC, b=BL),
                        Ident, bias=0.0, scale=1.0)

            def w2_matmul(j):
                # pre2 of chunk j -> pu tile of chunk j+LAG, l=1
                for gpair in range(2):
                    ps = ps2.tile([128, 2 * CB], f32, name=f"p2ps_{j}_{gpair}", tag="p2ps")
                    for gi in range(2):
                        g = 2 * gpair + gi
                        for c in range(4):
                            lhsT = w2_sb[:, (c * 4 + g) * 128:(c * 4 + g + 1) * 128]
                            nc.tensor.matmul(ps[:, gi * CB:(gi + 1) * CB], lhsT,
                                             mm_rhs(j, c),
                                             start=(c == 0),
                                             stop=(c == 3 and not with_b2))
                        if with_b2:
                            nc.tensor.matmul(ps[:, gi * CB:(gi + 1) * CB],
                                             b2_sb[0:1, g * 128:(g + 1) * 128],
                                             ones_sb[0:1, :],
                                             start=False, stop=True)
                    nc.scalar.activation(
                        p_drain_ap(j + LAG, 1, gpair), ps[:].rearrange(
                            "p (c t b) -> p c t b", c=2, t=TC, b=BL),
                        Ident, bias=0.0, scale=1.0)

            p1_matmul(0)
            for sb, dr in ((b2_sb, b2_d), (iu2_sb, iu2_d), (wf_sb, wf_d),
                           (w2_sb, w2_d)):
                nc.gpsimd.dma_start(sb[:], dr[:])
            if nchunks > 1:
                p1_matmul(1)

            zview = zst[:].rearrange("p (c s) -> p c s", c=4, s=SLOT)

            for k in range(nk):
                if 1 <= k <= nchunks:
                    w2_matmul(k - 1)
                if k + 2 < nchunks:
                    p1_matmul(k + 2)

                l2_active = k >= LAG
                l1_active = k < nchunks
                st_tiles[k] = st_pool.tile([128, ST_F], f16, name=f"st_{k}", tag="st")

                for i in range(TC):
                    if l1_active and l2_active:
                        if i == 0 and k == LAG:
                            _emit_step(nc, op, st_slot(k, 0, 0),
                                       st_slot(k - 1, TC - 1, 0), pu_slot(k, 0, 0))
                            _emit_step(nc, op, st_slot(k, 0, 1),
                                       zview, pu_slot(k, 0, 1))
                        else:
                            prev = (st_slot(k - 1, TC - 1) if i == 0
                                    else st_slot(k, i - 1))
                            _emit_step(nc, op, st_slot(k, i), prev, pu_slot(k, i))
                    elif l1_active:
                        if i == 0 and k == 0:
                            prev = zview
                        elif i == 0:
                            prev = st_slot(k - 1, TC - 1, 0)
                        else:
                            prev = st_slot(k, i - 1, 0)
                        _emit_step(nc, op, st_slot(k, i, 0), prev, pu_slot(k, i, 0))
                    else:
                        if i == 0:
                            prev = st_slot(k - 1, TC - 1, 1)
                        else:
                            prev = st_slot(k, i - 1, 1)
                        _emit_step(nc, op, st_slot(k, i, 1), prev, pu_slot(k, i, 1))

            # final: w2 = hi + lo of last layer-2 state; h2T = w2 * (1/u2)
            last = st_tiles[nk - 1][:].rearrange(
                "p (l c t b two) -> p l c t b two", l=2, c=4, t=TC, b=BL, two=2)
            w2f = misc.tile([128, 128], f32, name="w2f")
            nc.vector.tensor_tensor(
                w2f[:].rearrange("p (c b) -> p c b", c=4, b=BL),
                last[:, 1, :, TC - 1, :, 0], last[:, 1, :, TC - 1, :, 1], Add)
            hT = misc.tile([128, 128], f32r, name="hT")
            nc.vector.tensor_tensor(hT[:], w2f[:], iu2_sb[:], Mult)
            finps = ps2.tile([128, 2 * CB], f32, name="finps", tag="p2ps")
            fin = finps[0:1, 0:BL]
            for g_hi in range(4):
                nc.tensor.matmul(fin, wf_sb[:, g_hi:g_hi + 1],
                                 hT[:, g_hi * BL:(g_hi + 1) * BL],
                                 start=(g_hi == 0), stop=(g_hi == 3))
            out_sb = misc.tile([1, BL], f32, name="out_sb")
            nc.scalar.activation(out_sb[:], fin, Ident, bias=0.0, scale=1.0)
            nc.gpsimd.dma_start(out_d[:], out_sb[:])

    nc.compile()
    return nc


def _prep_inputs(x, W1, u1, b1, W2, u2, b2, Wf, bf, T):
    f = np.float32
    u1c = np.where(np.abs(u1) < 1e-6, np.where(u1 >= 0, 1e-6, -1e-6), u1).astype(f)
    u2c = np.where(np.abs(u2) < 1e-6, np.where(u2 >= 0, 1e-6, -1e-6), u2).astype(f)
    # compensate the mean of the hi-half truncation (hi = trunc_bf16(w)) seen
    # by the pre2 matmul: E[w - hi] ~ 2^-9 |w|
    W2p = ((W2 / u1c[None, :]) * (1.0 + 2.0 ** -9)).astype(f)

    nch = T // TC
    w1_rep = np.concatenate([W1.T, b1[None, :]], 0).astype(np.float16)  # [3, 512]
    b2_row = b2[None, :].astype(np.float16)                             # [1, 512]
    w2t = np.empty((128, 2048), np.float16)
    for c_hi in range(4):
        for g_hi in range(4):
            blk = W2p[g_hi * 128:(g_hi + 1) * 128, c_hi * 128:(c_hi + 1) * 128]
            w2t[:, (c_hi * 4 + g_hi) * 128:(c_hi * 4 + g_hi + 1) * 128] = blk.T
    wf_col = np.ascontiguousarray(Wf[0].reshape(4, 128).T).astype(f)
    iu2 = np.ascontiguousarray(
        np.broadcast_to((1.0 / u2c).reshape(4, 128).T[:, :, None],
                        (128, 4, BL)).reshape(128, 128)).astype(f)

    # pu init pattern [128, (l, c, t, 66)]: [u_hi, u_dlt, 32 x (p=0, junk=0)]
    pu = np.zeros((128, 2, 4, TC, BLK), np.float16)
    for lsel, uv in ((0, u1c), (1, u2c)):
        ucol = uv.reshape(4, 128).T                       # [c_lo, c_hi]
        uhi = ucol.astype(np.float16)
        udl = (ucol - uhi.astype(f)).astype(np.float16)
        pu[:, lsel, :, :, 0] = uhi[:, :, None]
        pu[:, lsel, :, :, 1] = udl[:, :, None]
    pu_init = np.ascontiguousarray(pu.reshape(128, 2 * 4 * TC * BLK))

    in_maps = []
    for core in range(NCORES):
        xs = x[core * BL:(core + 1) * BL, :T, :]
        x2 = xs.reshape(BL, nch, TC, 2).transpose(1, 3, 2, 0)     # [nch, 2, TC, BL]
        x_sb = np.empty((nch, 3, TC * BL), np.float16)
        x_sb[:, 0:2] = x2.reshape(nch, 2, TC * BL)
        x_sb[:, 2] = 1.0
        x_sb = np.ascontiguousarray(x_sb.reshape(3 * nch, TC * BL))
        in_maps.append({
            "x_sb": x_sb, "w1_rep": w1_rep, "pu_init": pu_init,
            "w2t": w2t, "b2_row": b2_row, "inv_u2": iu2, "wf_col": wf_col,
        })
    return in_maps


def kernel(x, W1, u1, b1, W2, u2, b2, Wf, bf, _T=None, _trace=False):
    x = np.asarray(x, np.float32)
    W1 = np.asarray(W1, np.float32); u1 = np.asarray(u1, np.float32)
    b1 = np.asarray(b1, np.float32); W2 = np.asarray(W2, np.float32)
    u2 = np.asarray(u2, np.float32); b2 = np.asarray(b2, np.float32)
    Wf = np.asarray(Wf, np.float32); bf = np.asarray(bf, np.float32)
    T = _T or x.shape[1]

    from concourse.bass_utils import run_bass_kernel_spmd

    with_b2 = bool(np.any(b2))
    key = (T, with_b2)
    if key not in _COMPILED:
        _COMPILED[key] = _build(T, with_b2=with_b2)
    nc = _COMPILED[key]

    in_maps = _prep_inputs(x, W1, u1, b1, W2, u2, b2, Wf, bf, T)
    res = run_bass_kernel_spmd(nc, in_maps, core_ids=list(range(NCORES)), trace=_trace)
    out = np.concatenate([res.results[i]["out"][0] for i in range(NCORES)]) + bf[0]
    kernel.last_exec_time_ns = res.exec_time_ns
    return out.astype(np.float32)
